# revision 43
# baseline (speedup 1.0000x reference)
"""Trainium2 Bass kernel for nn_AttentionBlock (GroupNorm + MHA + proj + residual).

Full inputs in, full output out. Sharding: 8 cores = 2 batches x 4 query-slices.
Each core: GroupNorm over its batch image (replicated within the batch group),
q projection for its 1024 queries, k/v projections over all 4096 keys,
per-head attention (S^T = k^T q formulation, softmax along the PSUM partition
axis via an appended ones-column in the PV matmul), output projection and
residual for its query slice. Host side only slices/rotates/concatenates.

v3: all GEMMs except S run in fp8e4 with DoubleRow perf mode (2 contraction
subtiles per matmul, 2x rate): q/k/v/proj projections contract 512 channels as
2 fp8 pairs; PV contracts 4096 keys as 16 m-tile pairs. The softmax exp runs
on ScalarE with scale=1/8 and bias=-SHIFT (keeps exp below fp8e4 max) and
writes fp8 directly into a 12-slot ring slab per head-stream; numerator and
denominator (ones-column) share the fp8 weights so the SHIFT cancels exactly.
S stays bf16 (64-deep contraction gains nothing from fp8) with the
swapped-row-half pairing so two heads' S matmuls co-run on disjoint PE rows.
v^T head blocks are padded to 66 cols (dual-fp8 ldweights needs even
cols/offsets): col 64 = ones (denominator), col 65 = zero pad.

Prologue: x loads via half-tile DMAs split across both hardware DMA queues
(in-order queues + DMA-ring flow control mean small/strided DMAs must go on
the gpsimd software queue and compute must interleave with issue streams);
weight transposes run on the PE before any attention matmul (in-order PE
queue), and all weight slab copies/normalize slices are engine-balanced.

ScalarE is the floor: ~2 exps of [128,1536] per period, 88 periods
(~265us of exp). Wall ~415us = ~85 prologue + ~310 stream + ~20 tail.
"""
import numpy as np

C = 512          # channels
N = 4096         # pixels (64*64)
NQ = 1024        # queries per core
H = 8            # heads
D = 64           # head dim
T = 4            # 128-channel chunks
W = NQ // 512    # query windows of 512
MT = N // 128    # key m-tiles of 128
NGROUPS = 8
EPS = 1e-5
GELEM = (C // NGROUPS) * N   # elements per norm group
NGRP = 11                    # m-groups per head stream: [3]*10 + [2]
SHIFT = 3.0                  # exp(s*0.125 - SHIFT): keeps exp < fp8e4 max
PRING = 12                   # pst ring slots (m-tiles); 12 = lcm-safe for
                             # 3-tile exp groups and 2-tile PV pairs
VX = 66                      # v^T head block: 64 dims + ones + pad

_COMPILED = None


def _emit(tc, io):
    import concourse.bass as bass
    from concourse import mybir
    from contextlib import ExitStack

    nc = tc.nc
    f32 = mybir.dt.float32
    bf16 = mybir.dt.bfloat16
    fp8 = mybir.dt.float8e4
    DR = mybir.MatmulPerfMode.DoubleRow
    Alu = mybir.AluOpType
    Act = mybir.ActivationFunctionType

    xb, qkvw, qkvb, projw, projb, nw, nb, y = (
        io["xb"], io["qkvw"], io["qkvb"], io["projw"], io["projb"],
        io["nw"], io["nb"], io["y"])

    ctx = ExitStack()
    with ctx:
        # ---------------- pools ----------------
        # PSUM: pool A (3 banks) = even-head S stream, pool B (3 banks) =
        # odd-head S stream, pv pool 2x1 bank. 3+3+2 = 8 banks. Phase 1/3/5
        # transposes/projection chains borrow A/B between attention uses.
        left = ctx.enter_context(tc.tile_pool(name="left", bufs=1))
        psum_a = ctx.enter_context(tc.tile_pool(name="psum_a", bufs=1, space="PSUM"))
        psum_b = ctx.enter_context(tc.tile_pool(name="psum_b", bufs=1, space="PSUM"))
        psum_pv = ctx.enter_context(tc.tile_pool(name="psum_pv", bufs=2, space="PSUM"))
        pool_ab = [psum_a, psum_b]

        right_ctx = ExitStack()
        xf_pool = right_ctx.enter_context(
            tc.tile_pool(name="xf_pool", bufs=1, side="right"))
        wstg_pool = right_ctx.enter_context(
            tc.tile_pool(name="wstg_pool", bufs=12, side="right"))
        wstg2_pool = ctx.enter_context(tc.tile_pool(name="wstg2_pool", bufs=4))
        scr_pool = right_ctx.enter_context(
            tc.tile_pool(name="scr_pool", bufs=2, side="right"))

        # ---------------- persistent tiles ----------------
        # xn8: fp8 normalized x, chunk-major slab [128, chunk(4) x N]
        xn8 = left.tile([128, T * N], fp8, name="xn8", tag="xn8")
        ksb = [left.tile([128, N], bf16, name=f"ksb{t}", tag=f"ksb{t}") for t in range(T)]
        qsb = [left.tile([128, NQ], bf16, name=f"qsb{t}", tag=f"qsb{t}") for t in range(T)]
        # wT8q: fp8 transposed qkv weights, [128, chunk(4) x 1536]
        wT8q = left.tile([128, T * 1536], fp8, name="wT8q", tag="wT8q")
        wT8p = left.tile([128, T * C], fp8, name="wT8p", tag="wT8p")
        ebias = left.tile([128, 1], f32, name="ebias", tag="ebias")
        qb = [left.tile([128, 1], f32, name=f"qb{i}", tag=f"qb{i}") for i in range(8)]
        vbp = [left.tile([128, 1], f32, name=f"vbp{i}", tag=f"vbp{i}") for i in range(T)]
        pb = [left.tile([128, 1], f32, name=f"pb{i}", tag=f"pb{i}") for i in range(T)]
        nwt = [left.tile([128, 1], f32, name=f"nwt{t}", tag=f"nwt{t}") for t in range(T)]
        nbt = [left.tile([128, 1], f32, name=f"nbt{t}", tag=f"nbt{t}") for t in range(T)]
        stat = [left.tile([128, 2], f32, name=f"stat{t}", tag=f"stat{t}") for t in range(T)]
        xres = [left.tile([128, NQ], bf16, name=f"xres{t}", tag=f"xres{t}") for t in range(T)]
        gstat = [left.tile([128, 2], f32, name=f"gstat{t}", tag=f"gstat{t}") for t in range(T)]

        # ---------------- weight DMAs + transposes FIRST ----------------
        # the PE queue is in-order: every weight transpose precedes the
        # attention matmuls, so weights must land before the x bulk load.
        xf = [xf_pool.tile([128, N], f32, name=f"xf{t}", tag=f"xf{t}") for t in range(T)]
        ident = left.tile([128, 128], f32, name="ident", tag="ident")
        nc.sync.dma_start(ident[:], io["cid"][:, :])
        ind = left.tile([128, 2], f32, name="ind", tag="ind")
        nc.gpsimd.dma_start(ind[:], io["cind"][:, :])
        indT = left.tile([2, 128], f32, name="indT", tag="indT")
        nc.gpsimd.dma_start(indT[0:2, :], io["cindT"][:, :])

        wstg_t = {}

        def prep_w_dma(i, src, eng):
            wstg = wstg_pool.tile([128, C], f32, name="wstg", tag="wstg")
            eng.dma_start(wstg[:], src[128 * i:128 * (i + 1), :])
            wstg_t[i] = wstg

        def prep_w(i, dst, stride):
            # transpose the 4 chan-chunk blocks (f32, PE idle) into one psum
            # bank, strided fp8 cast-copy into the slab (DVE/ScalarE split)
            wstg = wstg_t[i]
            tp = pool_ab[i % 2].tile([128, C], f32, name="tp",
                                     tag="sA" if i % 2 == 0 else "sB")
            for j in range(T):
                nc.tensor.transpose(tp[:, 128 * j:128 * (j + 1)],
                                    wstg[:, 128 * j:128 * (j + 1)], ident[:])
            dstv = bass.AP(tensor=dst[:].tensor, offset=dst[:].offset + 128 * i,
                           ap=[dst[:].ap[0], [stride, T], [1, 128]])
            srcv = tp[:].rearrange("p (c o) -> p c o", c=T)
            if i % 2 == 0:
                nc.vector.tensor_copy(dstv, srcv)
            else:
                nc.scalar.activation(dstv, srcv, Act.Copy)

        # norm/bias scalars on the gpsimd software queue: 24 tiny strided
        # DMAs would stall the scalar HW queue ahead of its x share
        for t in range(T):
            nc.gpsimd.dma_start(nwt[t][:, 0:1], nw[128 * t:128 * (t + 1)])
            nc.gpsimd.dma_start(nbt[t][:, 0:1], nb[128 * t:128 * (t + 1)])
            nc.gpsimd.dma_start(pb[t][:, 0:1], projb[128 * t:128 * (t + 1)])
            nc.gpsimd.dma_start(vbp[t][:, 0:1], qkvb[1024 + 128 * t:1024 + 128 * (t + 1)])
        for i in range(8):
            nc.gpsimd.dma_start(qb[i][:, 0:1], qkvb[128 * i:128 * (i + 1)])
        nc.vector.memset(ebias[:], -SHIFT)

        # ---------------- input DMAs: x bulk load FIRST ----------------
        # x gates stats->normalize->everything; weights gate only the
        # projection chains, so x gets all early HBM bandwidth. Half-tile
        # DMAs keep the per-partition descriptor at 8KB. Weight DMAs all on
        # sync (the scalar queue must reach its Squares promptly - DMA ring
        # flow control blocks the in-order queue).
        for t in range(T):
            nc.sync.dma_start(
                xf[t][:, 0:2048], xb[128 * t:128 * (t + 1), 0:2048])
        for t in range(2):
            nc.scalar.dma_start(
                xf[t][:, 2048:4096], xb[128 * t:128 * (t + 1), 2048:4096])

        # ---------------- phase 1: group stats ----------------
        # chunked: reduce/square each 1024-col chunk as its DMA lands, so the
        # stats pipeline overlaps the x load instead of serializing after it
        # spart cols: (sum_h0, sum_h1, sq_h0, sq_h1) per half-tile; the
        # indicator matmul both group-reduces partitions AND sums the 4
        # partials (gg[g] = sum over ch of halves) in one shot
        spart = [left.tile([128, 4], f32, name=f"spart{t}", tag=f"spart{t}")
                 for t in range(T)]
        ab_t = []
        for t in range(T):
            if t == 1:
                nc.scalar.dma_start(
                    xf[2][:, 2048:4096], xb[256:384, 2048:4096])
            if t == 2:
                nc.scalar.dma_start(
                    xf[3][:, 2048:4096], xb[384:512, 2048:4096])
                for i in (4, 5, 6, 7, 0, 1):
                    prep_w_dma(i, qkvw, nc.sync)
            for c2 in range(2):
                nc.vector.tensor_reduce(
                    out=spart[t][:, c2:c2 + 1],
                    in_=xf[t][:, 2048 * c2:2048 * (c2 + 1)],
                    axis=mybir.AxisListType.X, op=Alu.add)
                sq_scr = scr_pool.tile([128, 2048], bf16, name="sq_scr", tag="sq_scr")
                nc.scalar.activation(
                    sq_scr[:], xf[t][:, 2048 * c2:2048 * (c2 + 1)],
                    Act.Square, accum_out=spart[t][:, 2 + c2:3 + c2])
        # remaining weight tiles on the scalar queue - emitted after the
        # Squares so its in-order stream reaches them first; transfers run
        # after scalar's x share (~25us) while sync still streams its six
        for i in (8, 9, 10, 11, 2, 3):
            prep_w_dma(i, qkvw, nc.scalar)

        for t in range(T):
            gg_ps = psum_a.tile([2, 4], f32, name="gg_ps", tag="sA")
            nc.tensor.matmul(gg_ps[0:2, :], ind[:, 0:2], spart[t][:, 0:4],
                             start=True, stop=True)
            # fold the half-adds: gg2[g, s] = gg[g, 2s] + gg[g, 2s+1]
            gg_sb = left.tile([2, 4], f32, name=f"gg_sb{t}", tag=f"gg_sb{t}")
            nc.vector.tensor_copy(gg_sb[0:2, 0:4], gg_ps[0:2, :])
            gg2 = left.tile([2, 2], f32, name=f"gg2_{t}", tag=f"gg2_{t}")
            ev = bass.AP(tensor=gg_sb[:].tensor, offset=gg_sb[:].offset,
                         ap=[gg_sb[:].ap[0], [2, 2]])
            od = bass.AP(tensor=gg_sb[:].tensor, offset=gg_sb[:].offset + 1,
                         ap=[gg_sb[:].ap[0], [2, 2]])
            nc.vector.tensor_tensor(gg2[0:2, 0:2], ev, od, Alu.add)
            gb_ps = psum_b.tile([128, 2], f32, name="gb_ps", tag="sB")
            nc.tensor.matmul(gb_ps[:, 0:2], indT[0:2, :], gg2[0:2, 0:2],
                             start=True, stop=True)
            # ms = (mean, E[x^2]) in one scaled copy from PSUM
            ms = left.tile([128, 2], f32, name=f"ms{t}", tag=f"ms{t}")
            inv = 1.0 / GELEM
            nc.vector.tensor_scalar(ms[:, 0:2], gb_ps[:, 0:2], inv, None,
                                    Alu.mult)
            var_t = left.tile([128, 1], f32, name=f"var{t}", tag=f"var{t}")
            std_t = left.tile([128, 1], f32, name=f"std{t}", tag=f"std{t}")
            a_t = left.tile([128, 1], f32, name=f"a{t}", tag=f"a{t}")
            b_t = left.tile([128, 1], f32, name=f"b{t}", tag=f"b{t}")
            nc.vector.scalar_tensor_tensor(
                var_t[:], ms[:, 0:1], -1.0, ms[:, 0:1], Alu.mult, Alu.mult)
            nc.vector.scalar_tensor_tensor(
                var_t[:], ms[:, 1:2], EPS, var_t[:], Alu.add, Alu.add)
            nc.scalar.activation(std_t[:], var_t[:], Act.Sqrt)
            nc.vector.reciprocal(a_t[:], std_t[:])
            nc.vector.tensor_tensor(a_t[:], a_t[:], nwt[t][:], Alu.mult)
            nc.vector.tensor_tensor(b_t[:], ms[:, 0:1], a_t[:], Alu.mult)
            nc.vector.tensor_tensor(b_t[:], nbt[t][:], b_t[:], Alu.subtract)
            ab_t.append((a_t, b_t))

        wstg2_t = []
        for i in range(4):
            wstg = wstg2_pool.tile([128, C], f32, name="wstg2", tag="wstg2")
            nc.gpsimd.dma_start(wstg[:], projw[128 * i:128 * (i + 1), :])
            wstg2_t.append(wstg)

        def prep_w2(i):
            # proj weight row-tile (needed by the proj chains at period ~45)
            wstg = wstg2_t[i]
            tp = pool_ab[i % 2].tile([128, C], f32, name="tp2",
                                     tag="sA" if i % 2 == 0 else "sB")
            for j in range(T):
                nc.tensor.transpose(tp[:, 128 * j:128 * (j + 1)],
                                    wstg[:, 128 * j:128 * (j + 1)], ident[:])
            dstv = bass.AP(tensor=wT8p[:].tensor, offset=wT8p[:].offset + 128 * i,
                           ap=[wT8p[:].ap[0], [C, T], [1, 128]])
            srcv = tp[:].rearrange("p (c o) -> p c o", c=T)
            nc.vector.tensor_copy(dstv, srcv)

        # phase 2: normalize + fp8 cast, COLUMN halves: cols 0-2047 of all
        # chunks first so the w8<4 projection chains start while cols 2048+
        # still normalize. Engines round-robin; gpsimd takes late slices.
        for c2 in range(2):
            for t in range(T):
                a_t, b_t = ab_t[t]
                sl = slice(N * t + 2048 * c2, N * t + 2048 * (c2 + 1))
                eng = [nc.vector, nc.scalar, nc.vector, nc.scalar,
                       nc.vector, nc.scalar, nc.gpsimd, nc.gpsimd][4 * c2 + t]
                if eng is nc.scalar:
                    nc.scalar.activation(
                        xn8[:, sl], xf[t][:, 2048 * c2:2048 * (c2 + 1)],
                        Act.Identity, scale=a_t[:, 0:1], bias=b_t[:, 0:1])
                else:
                    eng.tensor_scalar(
                        xn8[:, sl], xf[t][:, 2048 * c2:2048 * (c2 + 1)],
                        a_t[:, 0:1], b_t[:, 0:1], Alu.mult, Alu.add)
        for t in range(T):
            # residual slice in bf16 via SBUF->SBUF casting DMA (idle engine)
            nc.gpsimd.dma_start(out=xres[t][:], in_=xf[t][:, 0:NQ])

        for i in (4, 8, 5, 9, 6, 10, 7, 11, 0, 2, 1, 3):
            prep_w(i, wT8q, 1536)

        right_ctx.close()

        # ---------------- mid pools (reuse xf space) ----------------
        mid = ctx.enter_context(tc.tile_pool(name="mid", bufs=1))
        psb_pool = ctx.enter_context(tc.tile_pool(name="psb_pool", bufs=4))
        rec_pool = ctx.enter_context(tc.tile_pool(name="rec_pool", bufs=2))
        yo_pool = ctx.enter_context(tc.tile_pool(name="yo_pool", bufs=2))

        vT = mid.tile([128, MT * (H * VX)], fp8, name="vT", tag="vT")
        ones_row = mid.tile([1, D], f32, name="ones_row", tag="ones_row")
        nc.vector.memset(ones_row[0:1, :], 1.0)
        dup_pool = ctx.enter_context(tc.tile_pool(name="dup_pool", bufs=4))
        yh = [mid.tile([128, 512], f32, name=f"yh{i}", tag=f"yh{i}") for i in range(T)]
        attn8 = mid.tile([128, T * NQ], fp8, name="attn8", tag="attn8")


        # ones (denominator trick) + zero pad columns of the v^T head blocks
        ones_view = vT[:].rearrange("p (m h x) -> p m h x", m=MT, x=VX)[:, :, :, 64:65]
        nc.vector.memset(ones_view, 1.0)
        pad_view = vT[:].rearrange("p (m h x) -> p m h x", m=MT, x=VX)[:, :, :, 65:66]
        nc.vector.memset(pad_view, 0.0)

        # ---------------- phase 3: projections (fp8 DoubleRow) ----------
        def wq_pair(kk, col, ncol):
            # stationary pair view of wT8q: chunks 2kk,2kk+1, out-cols col..
            return bass.AP(
                tensor=wT8q[:].tensor,
                offset=wT8q[:].offset + 1536 * 2 * kk + col,
                ap=[wT8q[:].ap[0], [1536, 2], [1, ncol]])

        def wp_pair(kk, col, ncol):
            return bass.AP(
                tensor=wT8p[:].tensor,
                offset=wT8p[:].offset + C * 2 * kk + col,
                ap=[wT8p[:].ap[0], [C, 2], [1, ncol]])

        def xn_pair(kk, col, ncol):
            return bass.AP(
                tensor=xn8[:].tensor,
                offset=xn8[:].offset + N * 2 * kk + col,
                ap=[xn8[:].ap[0], [N, 2], [1, ncol]])

        def attn_pair(kk, col, ncol):
            return bass.AP(
                tensor=attn8[:].tensor,
                offset=attn8[:].offset + NQ * 2 * kk + col,
                ap=[attn8[:].ap[0], [NQ, 2], [1, ncol]])

        def q_chain(i, w, par, sc=False):
            qp = pool_ab[par].tile([128, 512], f32, name="qp",
                                   tag="sA" if par == 0 else "sB")
            for kk in range(2):
                nc.tensor.matmul(
                    qp[:], wq_pair(kk, 128 * i, 128), xn_pair(kk, 512 * w, 512),
                    start=(kk == 0), stop=(kk == 1), perf_mode=DR)
            if sc:
                nc.scalar.activation(qsb[i][:, 512 * w:512 * w + 512], qp[:],
                                     Act.Identity, bias=qb[i][:, 0:1])
            else:
                nc.vector.tensor_scalar(
                    qsb[i][:, 512 * w:512 * w + 512], qp[:], qb[i][:, 0:1],
                    None, Alu.add)

        def k_chain(i, w8, par, sc=False):
            kp = pool_ab[par].tile([128, 512], f32, name="kp",
                                   tag="sA" if par == 0 else "sB")
            for kk in range(2):
                nc.tensor.matmul(
                    kp[:], wq_pair(kk, 512 + 128 * i, 128),
                    xn_pair(kk, 512 * w8, 512),
                    start=(kk == 0), stop=(kk == 1), perf_mode=DR)
            if sc:
                nc.scalar.activation(ksb[i][:, 512 * w8:512 * w8 + 512], kp[:],
                                     Act.Identity, bias=qb[4 + i][:, 0:1])
            else:
                nc.vector.tensor_scalar(
                    ksb[i][:, 512 * w8:512 * w8 + 512], kp[:], qb[4 + i][:, 0:1],
                    None, Alu.add)

        def v_chain2(mt, par):
            # two m-tiles per psum alloc: halves the pool round-trips that
            # serialize the early-period v drain against the S stream
            vp = pool_ab[par].tile([128, 1024], f32, name="vp2",
                                   tag="sA" if par == 0 else "sB")
            for sub in range(2):
                for kk in range(2):
                    nc.tensor.matmul(
                        vp[:, 512 * sub:512 * sub + 512],
                        xn_pair(kk, 128 * (mt + sub), 128),
                        wq_pair(kk, 1024, 512),
                        start=(kk == 0), stop=(kk == 1), perf_mode=DR)
            dst = vT[:, H * VX * mt:H * VX * (mt + 2)].rearrange(
                "p (m h x) -> p m h x", m=2, x=VX)[:, :, :, 0:64]
            srcv = vp[:].rearrange("p (m h x) -> p m h x", m=2, x=64)
            nc.vector.tensor_copy(dst, srcv)

        def v_chain(mt, par, sc=False):
            # out vp[128 keys, 512 vchan]; bias deferred to emit_normalize
            vp = pool_ab[par].tile([128, 512], f32, name="vp",
                                   tag="sA" if par == 0 else "sB")
            for kk in range(2):
                nc.tensor.matmul(
                    vp[:], xn_pair(kk, 128 * mt, 128), wq_pair(kk, 1024, 512),
                    start=(kk == 0), stop=(kk == 1), perf_mode=DR)
            dst = vT[:, H * VX * mt:H * VX * (mt + 1)].rearrange(
                "p (h x) -> p h x", x=VX)[:, :, 0:64]
            srcv = vp[:].rearrange("p (h x) -> p h x", x=64)
            # pure cast; prefix v casts may use ScalarE (idle pre-stream)
            if sc:
                nc.scalar.activation(dst, srcv, Act.Copy)
            else:
                nc.vector.tensor_copy(dst, srcv)

        def proj_chain(i, w, par, kks=(0, 1), chunks=None, partial=None,
                       combine=None, combine2=None):
            py = pool_ab[par].tile([128, 512], f32, name="py",
                                   tag="sA" if par == 0 else "sB")
            if chunks is None:
                for n_, kk in enumerate(kks):
                    nc.tensor.matmul(
                        py[:], wp_pair(kk, 128 * i, 128),
                        attn_pair(kk, 512 * w, 512),
                        start=(n_ == 0), stop=(n_ == len(kks) - 1), perf_mode=DR)
            else:
                # single-chunk fp8 matmuls (contraction 128 each, no DR)
                for n_, ck in enumerate(chunks):
                    nc.tensor.matmul(
                        py[:], wT8p[:, C * ck + 128 * i:C * ck + 128 * i + 128],
                        attn8[:, NQ * ck + 512 * w:NQ * ck + 512 * w + 512],
                        start=(n_ == 0), stop=(n_ == len(chunks) - 1))
            if partial is not None:
                nc.vector.tensor_copy(partial[:], py[:])
                return
            yo = yo_pool.tile([128, 512], f32, name="yo", tag="yo")
            nc.vector.scalar_tensor_tensor(
                yo[:], py[:], pb[i][:, 0:1], xres[i][:, 512 * w:512 * w + 512],
                Alu.add, Alu.add)
            if combine is not None:
                nc.vector.tensor_tensor(yo[:], yo[:], combine[:], Alu.add)
            if combine2 is not None:
                nc.vector.tensor_tensor(yo[:], yo[:], combine2[:], Alu.add)
            nc.sync.dma_start(y[128 * i:128 * i + 128, 512 * w:512 * w + 512], yo[:])

        # prefix: k tile 0 (all cols; dup half1 needs it by period 5),
        # k tile 1 cols 0-2047, window-0 q; the rest stream as fillers
        for w8 in range(4):
            k_chain(0, w8, w8 % 2)
            k_chain(1, w8, (w8 + 1) % 2)
        for w8 in range(4, 8):
            k_chain(0, w8, w8 % 2)
        for i in range(T):
            q_chain(i, 0, i % 2)

        # ---------------- phase 4: attention (flat pipelined stream) ------
        # Global stream of periods over (window, pair, group). PV runs one
        # period behind S/exp; pair normalize is deferred into the next
        # pair's first period; filler chains (k tiles 1-3, window-1 q,
        # window-0 proj) are emitted on alternate periods.
        def gsize(r):
            return 3 if r < NGRP - 1 else MT - 3 * (NGRP - 1)

        periods = [(w, p, r) for w in range(W) for p in range(4)
                   for r in range(NGRP)]
        pair_state = {}
        dup_state = {}

        def prep_dup(w, p, half):
            # swapped-row copies: kdup/qdup rows 64-127 hold head h0's data,
            # rows 0-63 hold h1's, so alternate S matmuls can run on disjoint
            # PE row halves (row-tiling concurrency). Pure relocation - no
            # numerical change. SBUF->SBUF DMA on otherwise idle engines.
            # Split in column halves so each DMA is emitted strictly after
            # the filler chains producing its source columns.
            if half == 0:
                kd = dup_pool.tile([128, N], bf16, name="kdup", tag="kdup")
                qd = dup_pool.tile([128, NQ], bf16, name="qdup", tag="qdup")
                dup_state[(w, p)] = (kd, qd)
                # q: only this pair's window columns are ever read
                nc.sync.dma_start(qd[64:128, 512 * w:512 * w + 512],
                                  qsb[p][0:64, 512 * w:512 * w + 512])
                nc.sync.dma_start(qd[0:64, 512 * w:512 * w + 512],
                                  qsb[p][64:128, 512 * w:512 * w + 512])
                lo, hi = 0, 2048
            else:
                kd, qd = dup_state[(w, p)]
                lo, hi = 2048, N
            nc.sync.dma_start(kd[64:128, lo:hi], ksb[p][0:64, lo:hi])
            nc.sync.dma_start(kd[0:64, lo:hi], ksb[p][64:128, lo:hi])

        def jmax(r):
            # last complete m-PAIR after exp group r: pairs j with 2j+1 <= 3r+2
            return (3 * r + 1) // 2

        def emit_pv(w, p, r):
            # fp8 DoubleRow: one matmul per m-PAIR (2 key-tiles, 2x rate)
            pvs, ps_t = pair_state[(w, p)]
            if pvs[0] is None:
                for hh in range(2):
                    pvs[hh] = psum_pv.tile([128, 512], f32, name=f"pv{hh}", tag="pv")
            j0 = jmax(r - 1) + 1 if r > 0 else 0
            for hh in range(2):
                h = 2 * p + hh
                pst = ps_t[hh]
                for j in range(j0, jmax(r) + 1):
                    s = (2 * j) % PRING
                    vpair = vT[:, 2 * H * VX * j:2 * H * VX * (j + 1)].rearrange(
                        "p (two h x) -> p two h x", two=2, x=VX)[:, :, h, :]
                    ppair = pst[:, 512 * s:512 * s + 1024].rearrange(
                        "p (two n) -> p two n", two=2)
                    nc.tensor.matmul(
                        pvs[hh][0:VX, :], vpair, ppair,
                        start=(j == 0), stop=(j == MT // 2 - 1),
                        perf_mode=DR)

        def emit_normalize(w, p, last=False):
            pvs, _ = pair_state[(w, p)]
            for hh in range(2):
                h = 2 * p + hh
                kt, prr = h // 2, 64 * (h % 2)
                pvc = rec_pool.tile([65, 512], f32, name="pvc", tag="pvc")
                nc.vector.tensor_copy(pvc[0:65, :], pvs[hh][0:65, :])
                dnm = rec_pool.tile([1, 512], f32, name="dnm", tag="dnm")
                nc.vector.tensor_copy(dnm[0:1, :], pvc[64:65, :])
                rec = rec_pool.tile([1, 512], f32, name="rec", tag="rec")
                rscr = rec_pool.tile([1, 512], f32, name="rscr", tag="rscr")
                nc.vector.reciprocal_approx_accurate(
                    rec[0:1, :], dnm[0:1, :], rscr[0:1, :])
                if last:
                    # tail: PE one-row broadcast (gpsimd drain costs ~4us)
                    bps = psum_pv.tile([64, 512], f32, name="bps", tag="pv")
                    nc.tensor.matmul(bps[0:64, :], ones_row[0:1, 0:64],
                                     rec[0:1, :], start=True, stop=True)
                    bcs = bps
                else:
                    # gpsimd partition-broadcast: idle engine, pure SBUF
                    bcs = rec_pool.tile([64, 512], f32, name="bcs", tag="bcs")
                    nc.gpsimd.partition_broadcast(bcs[0:64, :], rec[0:1, :])
                tmp = rec_pool.tile([64, 512], f32, name="tmp", tag="tmp")
                nc.vector.tensor_tensor(
                    tmp[0:64, :], pvc[0:64, :], bcs[0:64, :], Alu.mult)
                # + v-bias (per v-channel == partition here), cast fp8
                nc.vector.tensor_scalar(
                    attn8[prr:prr + 64, NQ * kt + 512 * w:NQ * kt + 512 * w + 512],
                    tmp[0:64, :], vbp[kt][prr:prr + 64, 0:1], None, Alu.add)

        # filler schedule: (earliest_period, closure); one pop on EVEN
        # periods, always from pool B (its next S alloc has ~2x more slack
        # than pool A's, so the filler's drain never delays the exp stream).
        # Emission deadlines: ksb[2] before period 22, ksb[3] before 33,
        # window-1 q before 44, window-0 proj after normalize(w0,p3) at 44.
        fillers = []
        for w8 in range(4, 8):                      # ksb[1] cols 2048+: by the
            fillers.append((1 + w8, lambda w8=w8:   # pair-1 dup at g=10
                            k_chain(1, w8, w8 % 2)))
        for i in range(2, T):                       # ksb[2..3]: pops 2..16, 18..32
            for w8 in range(8):
                fillers.append((16 * (i - 2) + 2 + 2 * w8,
                                lambda i=i, w8=w8: k_chain(i, w8, 1)))
        fillers.sort(key=lambda f: f[0])
        for i in range(T):                          # window-1 q: pops 34..40
            fillers.append((34 + 2 * i, lambda i=i: q_chain(i, 1, 1)))
        for i in range(T):                          # window-0 proj: pops 46..52
            fillers.append((46 + 2 * i, lambda i=i: proj_chain(i, 0, 1)))
        for i in range(T):                          # w1 proj half (pairs 0-1)
            fillers.append((70 + 2 * i, lambda i=i:
                            proj_chain(i, 1, 1, kks=(0,), partial=yh[i])))

        def pre_res(i):
            # yh[i] + proj bias + residual, pre-fused so the tail is one add
            nc.vector.scalar_tensor_tensor(
                yh[i][:], yh[i][:], pb[i][:, 0:1], xres[i][:, 512:1024],
                Alu.add, Alu.add)
        for i in range(T):
            fillers.append((80 + i, lambda i=i: pre_res(i)))
        fillers.reverse()   # pop from the end

        prep_dup(0, 0, 0)
        prep_dup(0, 0, 1)
        for mt in range(8):
            v_chain(mt, mt % 2)
        vq = list(range(8, MT, 2))  # v m-pairs: drained 2-per-period below
        for g, (w, p, r) in enumerate(periods):
            if g == 1:
                # proj weight prep: PE transposes must precede the first PV
                # matmul in the queue (transpose inside an open PV
                # accumulation group corrupts PSUM)
                for i_ in range(4):
                    prep_w2(i_)
            gs = gsize(r)
            if r == 0:
                pair_state[(w, p)] = (
                    [None, None],
                    [psb_pool.tile([128, 512 * PRING], fp8, name=f"ps{hh}",
                                   tag="ps") for hh in range(2)])
            pvs, ps_t = pair_state[(w, p)]
            kd, qd = dup_state[(w, p)]
            for hh in range(2):
                sp = pool_ab[hh].tile([128, 512 * gs], f32, name=f"sp{hh}",
                                      tag="sA" if hh == 0 else "sB")
                for j in range(gs):
                    m = 3 * r + j
                    # even j: head's native row half; odd j: the swapped copy
                    # on the opposite half -> adjacent matmuls use disjoint
                    # PE row groups and run concurrently
                    if j % 2 == 0:
                        pr, kt_, qt_ = 64 * hh, ksb[p], qsb[p]
                    else:
                        pr, kt_, qt_ = 64 * (1 - hh), kd, qd
                    nc.tensor.matmul(
                        sp[:, 512 * j:512 * j + 512],
                        kt_[pr:pr + 64, 128 * m:128 * m + 128],
                        qt_[pr:pr + 64, 512 * w:512 * w + 512],
                        start=True, stop=True)
                s0 = (3 * r) % PRING
                nc.scalar.activation(
                    ps_t[hh][:, 512 * s0:512 * s0 + 512 * gs],
                    sp[:, 0:512 * gs], Act.Exp, scale=0.125,
                    bias=ebias[:, 0:1])
                # PV of the previous period goes between the two S blocks
                # (h0 part) and after them (h1 part)
                if hh == 0:
                    if g > 0:
                        pw, pp, prr_ = periods[g - 1]
                        emit_pv(pw, pp, prr_)
            if g > 0 and periods[g - 1][2] == NGRP - 1:
                emit_normalize(*periods[g - 1][:2])
            for _ in range(2):
                # v(m,m+1) must land before PV pair m/2 fires at period
                # ceil((m-1)/3)+1
                if vq and g >= 1:
                    mt = vq.pop(0)
                    v_chain2(mt, (mt // 2) % 2)
                else:
                    break
            if fillers and fillers[-1][0] <= g:
                fillers.pop()[1]()
            if r == 5 and g + 6 < len(periods):
                prep_dup(*periods[g + 6][:2], 0)
            if r == NGRP - 1 and g + 1 < len(periods):
                prep_dup(*periods[g + 1][:2], 1)

        # tail: last period's PV, last normalize, window-1 proj second half
        emit_pv(*periods[-1])
        emit_normalize(W - 1, 3, last=True)
        while fillers:
            fillers.pop()[1]()
        for i in range(T):
            py = pool_ab[i % 2].tile([128, 512], f32, name="py",
                                     tag="sA" if i % 2 == 0 else "sB")
            nc.tensor.matmul(py[:], wp_pair(1, 128 * i, 128),
                             attn_pair(1, 512, 512),
                             start=True, stop=True, perf_mode=DR)
            yo = yo_pool.tile([128, 512], f32, name="yo", tag="yo")
            nc.vector.tensor_tensor(yo[:], py[:], yh[i][:], Alu.add)
            nc.sync.dma_start(y[128 * i:128 * i + 128, 512:1024], yo[:])


def _build():
    import concourse.tile as tile
    from concourse import bacc, mybir

    nc = bacc.Bacc("TRN2", target_bir_lowering=False, debug=False)
    f32 = mybir.dt.float32
    io = {
        "xb": nc.dram_tensor("xb", [C, N], f32, kind="ExternalInput").ap(),
        "qkvw": nc.dram_tensor("qkvw", [3 * C, C], f32, kind="ExternalInput").ap(),
        "qkvb": nc.dram_tensor("qkvb", [3 * C], f32, kind="ExternalInput").ap(),
        "projw": nc.dram_tensor("projw", [C, C], f32, kind="ExternalInput").ap(),
        "projb": nc.dram_tensor("projb", [C], f32, kind="ExternalInput").ap(),
        "nw": nc.dram_tensor("nw", [C], f32, kind="ExternalInput").ap(),
        "nb": nc.dram_tensor("nb", [C], f32, kind="ExternalInput").ap(),
        "cid": nc.dram_tensor("cid", [128, 128], mybir.dt.float32,
                              kind="ExternalInput").ap(),
        "cind": nc.dram_tensor("cind", [128, 2], f32, kind="ExternalInput").ap(),
        "cindT": nc.dram_tensor("cindT", [2, 128], f32, kind="ExternalInput").ap(),
        "y": nc.dram_tensor("y", [C, NQ], f32, kind="ExternalOutput").ap(),
    }
    with tile.TileContext(nc) as tc:
        _emit(tc, io)
    nc.compile()
    return nc


def get_compiled():
    global _COMPILED
    if _COMPILED is None:
        _COMPILED = _build()
    return _COMPILED


def make_in_maps(x, norm_w, norm_b, qkv_w, qkv_b, proj_w, proj_b):
    import ml_dtypes

    xf = np.ascontiguousarray(np.asarray(x, np.float32)).reshape(2, C, N)
    ind = np.zeros((128, 2), np.float32)
    ind[0:64, 0] = 1.0
    ind[64:128, 1] = 1.0
    shared = {
        "cid": np.eye(128, dtype=np.float32),
        "cind": ind,
        "cindT": np.ascontiguousarray(ind.T),
        "qkvw": np.ascontiguousarray(np.asarray(qkv_w, np.float32)),
        "qkvb": np.ascontiguousarray(np.asarray(qkv_b, np.float32)),
        "projw": np.ascontiguousarray(np.asarray(proj_w, np.float32)),
        "projb": np.ascontiguousarray(np.asarray(proj_b, np.float32)),
        "nw": np.ascontiguousarray(np.asarray(norm_w, np.float32)),
        "nb": np.ascontiguousarray(np.asarray(norm_b, np.float32)),
    }
    in_maps = []
    for core in range(8):
        bi, qs = core // 4, core % 4
        # rotate so this core's queries are always columns [0:NQ)
        xroll = np.concatenate(
            [xf[bi][:, qs * NQ:], xf[bi][:, :qs * NQ]], axis=1)
        m = dict(shared)
        m["xb"] = np.ascontiguousarray(xroll)
        in_maps.append(m)
    return in_maps


def assemble(results, x):
    y = np.zeros((2, C, N), np.float32)
    for core in range(8):
        bi, qs = core // 4, core % 4
        y[bi][:, qs * NQ:(qs + 1) * NQ] = results[core]["y"]
    return y.reshape(x.shape)


def kernel(x, norm_w, norm_b, qkv_w, qkv_b, proj_w, proj_b, **_ignored):
    from concourse import bass_utils

    nc = get_compiled()
    in_maps = make_in_maps(x, norm_w, norm_b, qkv_w, qkv_b, proj_w, proj_b)
    res = bass_utils.run_bass_kernel_spmd(nc, in_maps, core_ids=list(range(8)))
    return assemble(res.results, np.asarray(x))


# revision 44
# speedup vs baseline: 1.0190x; 1.0190x over previous
"""Trainium2 Bass kernel for nn_AttentionBlock (GroupNorm + MHA + proj + residual).

Full inputs in, full output out. Sharding: 8 cores = 2 batches x 4 query-slices.
Each core: GroupNorm over its batch image (replicated within the batch group),
q projection for its 1024 queries, k/v projections over all 4096 keys,
per-head attention (S^T = k^T q formulation, softmax along the PSUM partition
axis via an appended ones-column in the PV matmul), output projection and
residual for its query slice. Host side only slices/rotates/concatenates.

v3: all GEMMs except S run in fp8e4 with DoubleRow perf mode (2 contraction
subtiles per matmul, 2x rate): q/k/v/proj projections contract 512 channels as
2 fp8 pairs; PV contracts 4096 keys as 16 m-tile pairs. The softmax exp runs
on ScalarE with scale=1/8 and bias=-SHIFT (keeps exp below fp8e4 max) and
writes fp8 directly into a 12-slot ring slab per head-stream; numerator and
denominator (ones-column) share the fp8 weights so the SHIFT cancels exactly.
S stays bf16 (64-deep contraction gains nothing from fp8) with the
swapped-row-half pairing so two heads' S matmuls co-run on disjoint PE rows.
v^T head blocks are padded to 66 cols (dual-fp8 ldweights needs even
cols/offsets): col 64 = ones (denominator), col 65 = zero pad.

Prologue: x loads via half-tile DMAs split across both hardware DMA queues
(in-order queues + DMA-ring flow control mean small/strided DMAs must go on
the gpsimd software queue and compute must interleave with issue streams);
weight transposes run on the PE before any attention matmul (in-order PE
queue), and all weight slab copies/normalize slices are engine-balanced.

ScalarE is the floor: ~2 exps of [128,1536] per period, 88 periods
(~265us of exp). Wall ~415us = ~85 prologue + ~310 stream + ~20 tail.
"""
import numpy as np

C = 512          # channels
N = 4096         # pixels (64*64)
NQ = 1024        # queries per core
H = 8            # heads
D = 64           # head dim
T = 4            # 128-channel chunks
W = NQ // 512    # query windows of 512
MT = N // 128    # key m-tiles of 128
NGROUPS = 8
EPS = 1e-5
GELEM = (C // NGROUPS) * N   # elements per norm group
NGRP = 11                    # m-groups per head stream: [3]*10 + [2]
SHIFT = 3.0                  # exp(s*0.125 - SHIFT): keeps exp < fp8e4 max
PRING = 12                   # pst ring slots (m-tiles); 12 = lcm-safe for
                             # 3-tile exp groups and 2-tile PV pairs
VX = 66                      # v^T head block: 64 dims + ones + pad

_COMPILED = None


def _emit(tc, io):
    import concourse.bass as bass
    from concourse import mybir
    from contextlib import ExitStack

    nc = tc.nc
    f32 = mybir.dt.float32
    bf16 = mybir.dt.bfloat16
    fp8 = mybir.dt.float8e4
    DR = mybir.MatmulPerfMode.DoubleRow
    Alu = mybir.AluOpType
    Act = mybir.ActivationFunctionType

    xb, qkvw, qkvb, projw, projb, nw, nb, y = (
        io["xb"], io["qkvw"], io["qkvb"], io["projw"], io["projb"],
        io["nw"], io["nb"], io["y"])

    ctx = ExitStack()
    with ctx:
        # ---------------- pools ----------------
        # PSUM: pool A (3 banks) = even-head S stream, pool B (3 banks) =
        # odd-head S stream, pv pool 2x1 bank. 3+3+2 = 8 banks. Phase 1/3/5
        # transposes/projection chains borrow A/B between attention uses.
        left = ctx.enter_context(tc.tile_pool(name="left", bufs=1))
        psum_a = ctx.enter_context(tc.tile_pool(name="psum_a", bufs=1, space="PSUM"))
        psum_b = ctx.enter_context(tc.tile_pool(name="psum_b", bufs=1, space="PSUM"))
        psum_pv = ctx.enter_context(tc.tile_pool(name="psum_pv", bufs=2, space="PSUM"))
        pool_ab = [psum_a, psum_b]

        right_ctx = ExitStack()
        xf_pool = right_ctx.enter_context(
            tc.tile_pool(name="xf_pool", bufs=1, side="right"))
        wstg_pool = right_ctx.enter_context(
            tc.tile_pool(name="wstg_pool", bufs=12, side="right"))
        wstg2_pool = ctx.enter_context(tc.tile_pool(name="wstg2_pool", bufs=4))
        scr_pool = right_ctx.enter_context(
            tc.tile_pool(name="scr_pool", bufs=2, side="right"))

        # ---------------- persistent tiles ----------------
        # xn8: fp8 normalized x, chunk-major slab [128, chunk(4) x N]
        xn8 = left.tile([128, T * N], fp8, name="xn8", tag="xn8")
        ksb = [left.tile([128, N], bf16, name=f"ksb{t}", tag=f"ksb{t}") for t in range(T)]
        qsb = [left.tile([128, NQ], bf16, name=f"qsb{t}", tag=f"qsb{t}") for t in range(T)]
        # wT8q: fp8 transposed qkv weights, [128, chunk(4) x 1536]
        wT8q = left.tile([128, T * 1536], fp8, name="wT8q", tag="wT8q")
        wT8p = left.tile([128, T * C], fp8, name="wT8p", tag="wT8p")
        ebias = left.tile([128, 1], f32, name="ebias", tag="ebias")
        qb = [left.tile([128, 1], f32, name=f"qb{i}", tag=f"qb{i}") for i in range(8)]
        vbp = [left.tile([128, 1], f32, name=f"vbp{i}", tag=f"vbp{i}") for i in range(T)]
        pb = [left.tile([128, 1], f32, name=f"pb{i}", tag=f"pb{i}") for i in range(T)]
        nwt = [left.tile([128, 1], f32, name=f"nwt{t}", tag=f"nwt{t}") for t in range(T)]
        nbt = [left.tile([128, 1], f32, name=f"nbt{t}", tag=f"nbt{t}") for t in range(T)]
        stat = [left.tile([128, 2], f32, name=f"stat{t}", tag=f"stat{t}") for t in range(T)]
        xres = [left.tile([128, NQ], bf16, name=f"xres{t}", tag=f"xres{t}") for t in range(T)]
        gstat = [left.tile([128, 2], f32, name=f"gstat{t}", tag=f"gstat{t}") for t in range(T)]

        # ---------------- weight DMAs + transposes FIRST ----------------
        # the PE queue is in-order: every weight transpose precedes the
        # attention matmuls, so weights must land before the x bulk load.
        xf = [xf_pool.tile([128, N], f32, name=f"xf{t}", tag=f"xf{t}") for t in range(T)]
        ident = left.tile([128, 128], f32, name="ident", tag="ident")
        nc.sync.dma_start(ident[:], io["cid"][:, :])
        ind = left.tile([128, 2], f32, name="ind", tag="ind")
        nc.gpsimd.dma_start(ind[:], io["cind"][:, :])
        indT = left.tile([2, 128], f32, name="indT", tag="indT")
        nc.gpsimd.dma_start(indT[0:2, :], io["cindT"][:, :])

        wstg_t = {}

        def prep_w_dma(i, src, eng):
            wstg = wstg_pool.tile([128, C], f32, name="wstg", tag="wstg")
            eng.dma_start(wstg[:], src[128 * i:128 * (i + 1), :])
            wstg_t[i] = wstg

        def prep_w(i, dst, stride):
            # transpose the 4 chan-chunk blocks (f32, PE idle) into one psum
            # bank, strided fp8 cast-copy into the slab (DVE/ScalarE split)
            wstg = wstg_t[i]
            tp = pool_ab[i % 2].tile([128, C], f32, name="tp",
                                     tag="sA" if i % 2 == 0 else "sB")
            for j in range(T):
                nc.tensor.transpose(tp[:, 128 * j:128 * (j + 1)],
                                    wstg[:, 128 * j:128 * (j + 1)], ident[:])
            dstv = bass.AP(tensor=dst[:].tensor, offset=dst[:].offset + 128 * i,
                           ap=[dst[:].ap[0], [stride, T], [1, 128]])
            srcv = tp[:].rearrange("p (c o) -> p c o", c=T)
            if i % 2 == 0:
                nc.vector.tensor_copy(dstv, srcv)
            else:
                nc.scalar.activation(dstv, srcv, Act.Copy)

        # norm/bias scalars on the gpsimd software queue: 24 tiny strided
        # DMAs would stall the scalar HW queue ahead of its x share
        for t in range(T):
            nc.gpsimd.dma_start(nwt[t][:, 0:1], nw[128 * t:128 * (t + 1)])
            nc.gpsimd.dma_start(nbt[t][:, 0:1], nb[128 * t:128 * (t + 1)])
            nc.gpsimd.dma_start(pb[t][:, 0:1], projb[128 * t:128 * (t + 1)])
            nc.gpsimd.dma_start(vbp[t][:, 0:1], qkvb[1024 + 128 * t:1024 + 128 * (t + 1)])
        for i in range(8):
            nc.gpsimd.dma_start(qb[i][:, 0:1], qkvb[128 * i:128 * (i + 1)])
        nc.vector.memset(ebias[:], -SHIFT)

        # ---------------- input DMAs: x bulk load FIRST ----------------
        # x gates stats->normalize->everything; weights gate only the
        # projection chains, so x gets all early HBM bandwidth. Half-tile
        # DMAs keep the per-partition descriptor at 8KB. Weight DMAs all on
        # sync (the scalar queue must reach its Squares promptly - DMA ring
        # flow control blocks the in-order queue).
        for t in range(T):
            nc.sync.dma_start(
                xf[t][:, 0:2048], xb[128 * t:128 * (t + 1), 0:2048])
        for t in range(2):
            nc.scalar.dma_start(
                xf[t][:, 2048:4096], xb[128 * t:128 * (t + 1), 2048:4096])

        # ---------------- phase 1: group stats ----------------
        # chunked: reduce/square each 1024-col chunk as its DMA lands, so the
        # stats pipeline overlaps the x load instead of serializing after it
        # spart cols: (sum_h0, sum_h1, sq_h0, sq_h1) per half-tile; the
        # indicator matmul both group-reduces partitions AND sums the 4
        # partials (gg[g] = sum over ch of halves) in one shot
        spart = [left.tile([128, 4], f32, name=f"spart{t}", tag=f"spart{t}")
                 for t in range(T)]
        ab_t = []
        for t in range(T):
            if t == 1:
                nc.scalar.dma_start(
                    xf[2][:, 2048:4096], xb[256:384, 2048:4096])
            if t == 2:
                nc.scalar.dma_start(
                    xf[3][:, 2048:4096], xb[384:512, 2048:4096])
                for i in (4, 5, 6, 7, 0, 1):
                    prep_w_dma(i, qkvw, nc.sync)
            for c2 in range(2):
                nc.vector.tensor_reduce(
                    out=spart[t][:, c2:c2 + 1],
                    in_=xf[t][:, 2048 * c2:2048 * (c2 + 1)],
                    axis=mybir.AxisListType.X, op=Alu.add)
                sq_scr = scr_pool.tile([128, 2048], bf16, name="sq_scr", tag="sq_scr")
                nc.scalar.activation(
                    sq_scr[:], xf[t][:, 2048 * c2:2048 * (c2 + 1)],
                    Act.Square, accum_out=spart[t][:, 2 + c2:3 + c2])
        # remaining weight tiles on the scalar queue - emitted after the
        # Squares so its in-order stream reaches them first; transfers run
        # after scalar's x share (~25us) while sync still streams its six
        for i in (8, 9, 10, 11, 2, 3):
            prep_w_dma(i, qkvw, nc.scalar)

        for t in range(T):
            gg_ps = psum_a.tile([2, 4], f32, name="gg_ps", tag="sA")
            nc.tensor.matmul(gg_ps[0:2, :], ind[:, 0:2], spart[t][:, 0:4],
                             start=True, stop=True)
            # fold the half-adds: gg2[g, s] = gg[g, 2s] + gg[g, 2s+1]
            gg_sb = left.tile([2, 4], f32, name=f"gg_sb{t}", tag=f"gg_sb{t}")
            nc.vector.tensor_copy(gg_sb[0:2, 0:4], gg_ps[0:2, :])
            gg2 = left.tile([2, 2], f32, name=f"gg2_{t}", tag=f"gg2_{t}")
            ev = bass.AP(tensor=gg_sb[:].tensor, offset=gg_sb[:].offset,
                         ap=[gg_sb[:].ap[0], [2, 2]])
            od = bass.AP(tensor=gg_sb[:].tensor, offset=gg_sb[:].offset + 1,
                         ap=[gg_sb[:].ap[0], [2, 2]])
            nc.vector.tensor_tensor(gg2[0:2, 0:2], ev, od, Alu.add)
            gb_ps = psum_b.tile([128, 2], f32, name="gb_ps", tag="sB")
            nc.tensor.matmul(gb_ps[:, 0:2], indT[0:2, :], gg2[0:2, 0:2],
                             start=True, stop=True)
            # ms = (mean, E[x^2]) in one scaled copy from PSUM
            ms = left.tile([128, 2], f32, name=f"ms{t}", tag=f"ms{t}")
            inv = 1.0 / GELEM
            nc.vector.tensor_scalar(ms[:, 0:2], gb_ps[:, 0:2], inv, None,
                                    Alu.mult)
            var_t = left.tile([128, 1], f32, name=f"var{t}", tag=f"var{t}")
            std_t = left.tile([128, 1], f32, name=f"std{t}", tag=f"std{t}")
            a_t = left.tile([128, 1], f32, name=f"a{t}", tag=f"a{t}")
            b_t = left.tile([128, 1], f32, name=f"b{t}", tag=f"b{t}")
            nc.vector.scalar_tensor_tensor(
                var_t[:], ms[:, 0:1], -1.0, ms[:, 0:1], Alu.mult, Alu.mult)
            nc.vector.scalar_tensor_tensor(
                var_t[:], ms[:, 1:2], EPS, var_t[:], Alu.add, Alu.add)
            nc.scalar.activation(std_t[:], var_t[:], Act.Sqrt)
            nc.vector.reciprocal(a_t[:], std_t[:])
            nc.vector.tensor_tensor(a_t[:], a_t[:], nwt[t][:], Alu.mult)
            nc.vector.tensor_tensor(b_t[:], ms[:, 0:1], a_t[:], Alu.mult)
            nc.vector.tensor_tensor(b_t[:], nbt[t][:], b_t[:], Alu.subtract)
            ab_t.append((a_t, b_t))

        wstg2_t = []
        for i in range(4):
            wstg = wstg2_pool.tile([128, C], f32, name="wstg2", tag="wstg2")
            nc.gpsimd.dma_start(wstg[:], projw[128 * i:128 * (i + 1), :])
            wstg2_t.append(wstg)

        def prep_w2(i):
            # proj weight row-tile (needed by the proj chains at period ~45)
            wstg = wstg2_t[i]
            tp = pool_ab[i % 2].tile([128, C], f32, name="tp2",
                                     tag="sA" if i % 2 == 0 else "sB")
            for j in range(T):
                nc.tensor.transpose(tp[:, 128 * j:128 * (j + 1)],
                                    wstg[:, 128 * j:128 * (j + 1)], ident[:])
            dstv = bass.AP(tensor=wT8p[:].tensor, offset=wT8p[:].offset + 128 * i,
                           ap=[wT8p[:].ap[0], [C, T], [1, 128]])
            srcv = tp[:].rearrange("p (c o) -> p c o", c=T)
            nc.vector.tensor_copy(dstv, srcv)

        # phase 2: normalize + fp8 cast, COLUMN halves: cols 0-2047 of all
        # chunks first so the w8<4 projection chains start while cols 2048+
        # still normalize. Engines round-robin; gpsimd takes late slices.
        for c2 in range(2):
            for t in range(T):
                a_t, b_t = ab_t[t]
                sl = slice(N * t + 2048 * c2, N * t + 2048 * (c2 + 1))
                eng = [nc.vector, nc.scalar, nc.vector, nc.scalar,
                       nc.vector, nc.scalar, nc.gpsimd, nc.gpsimd][4 * c2 + t]
                if eng is nc.scalar:
                    nc.scalar.activation(
                        xn8[:, sl], xf[t][:, 2048 * c2:2048 * (c2 + 1)],
                        Act.Identity, scale=a_t[:, 0:1], bias=b_t[:, 0:1])
                else:
                    eng.tensor_scalar(
                        xn8[:, sl], xf[t][:, 2048 * c2:2048 * (c2 + 1)],
                        a_t[:, 0:1], b_t[:, 0:1], Alu.mult, Alu.add)
        for t in range(T):
            # residual slice in bf16 via SBUF->SBUF casting DMA (idle engine)
            nc.gpsimd.dma_start(out=xres[t][:], in_=xf[t][:, 0:NQ])

        for i in (4, 8, 5, 9, 6, 10, 7, 11, 0, 2, 1, 3):
            prep_w(i, wT8q, 1536)

        right_ctx.close()

        # ---------------- mid pools (reuse xf space) ----------------
        mid = ctx.enter_context(tc.tile_pool(name="mid", bufs=1))
        psb_pool = ctx.enter_context(tc.tile_pool(name="psb_pool", bufs=4))
        rec_pool = ctx.enter_context(tc.tile_pool(name="rec_pool", bufs=2))
        yo_pool = ctx.enter_context(tc.tile_pool(name="yo_pool", bufs=2))

        vT = mid.tile([128, MT * (H * VX)], fp8, name="vT", tag="vT")
        ones_row = mid.tile([1, D], f32, name="ones_row", tag="ones_row")
        nc.vector.memset(ones_row[0:1, :], 1.0)
        dup_pool = ctx.enter_context(tc.tile_pool(name="dup_pool", bufs=4))
        yh = [mid.tile([128, 512], f32, name=f"yh{i}", tag=f"yh{i}") for i in range(T)]
        attn8 = mid.tile([128, T * NQ], fp8, name="attn8", tag="attn8")


        # ones (denominator trick) + zero pad columns of the v^T head blocks
        ones_view = vT[:].rearrange("p (m h x) -> p m h x", m=MT, x=VX)[:, :, :, 64:65]
        nc.vector.memset(ones_view, 1.0)
        pad_view = vT[:].rearrange("p (m h x) -> p m h x", m=MT, x=VX)[:, :, :, 65:66]
        nc.vector.memset(pad_view, 0.0)

        # ---------------- phase 3: projections (fp8 DoubleRow) ----------
        def wq_pair(kk, col, ncol):
            # stationary pair view of wT8q: chunks 2kk,2kk+1, out-cols col..
            return bass.AP(
                tensor=wT8q[:].tensor,
                offset=wT8q[:].offset + 1536 * 2 * kk + col,
                ap=[wT8q[:].ap[0], [1536, 2], [1, ncol]])

        def wp_pair(kk, col, ncol):
            return bass.AP(
                tensor=wT8p[:].tensor,
                offset=wT8p[:].offset + C * 2 * kk + col,
                ap=[wT8p[:].ap[0], [C, 2], [1, ncol]])

        def xn_pair(kk, col, ncol):
            return bass.AP(
                tensor=xn8[:].tensor,
                offset=xn8[:].offset + N * 2 * kk + col,
                ap=[xn8[:].ap[0], [N, 2], [1, ncol]])

        def attn_pair(kk, col, ncol):
            return bass.AP(
                tensor=attn8[:].tensor,
                offset=attn8[:].offset + NQ * 2 * kk + col,
                ap=[attn8[:].ap[0], [NQ, 2], [1, ncol]])

        def q_chain(i, w, par, sc=False):
            qp = pool_ab[par].tile([128, 512], f32, name="qp",
                                   tag="sA" if par == 0 else "sB")
            for kk in range(2):
                nc.tensor.matmul(
                    qp[:], wq_pair(kk, 128 * i, 128), xn_pair(kk, 512 * w, 512),
                    start=(kk == 0), stop=(kk == 1), perf_mode=DR)
            if sc:
                nc.scalar.activation(qsb[i][:, 512 * w:512 * w + 512], qp[:],
                                     Act.Identity, bias=qb[i][:, 0:1])
            else:
                nc.vector.tensor_scalar(
                    qsb[i][:, 512 * w:512 * w + 512], qp[:], qb[i][:, 0:1],
                    None, Alu.add)

        def k_chain2(i, w8p, par):
            # two key-column blocks per psum alloc: halves the bias-cast ops
            # and pool round-trips (same mechanism as v_chain2)
            kp = pool_ab[par].tile([128, 1024], f32, name="kp2",
                                   tag="sA" if par == 0 else "sB")
            for sub in range(2):
                for kk in range(2):
                    nc.tensor.matmul(
                        kp[:, 512 * sub:512 * sub + 512],
                        wq_pair(kk, 512 + 128 * i, 128),
                        xn_pair(kk, 512 * (w8p + sub), 512),
                        start=(kk == 0), stop=(kk == 1), perf_mode=DR)
            nc.vector.tensor_scalar(
                ksb[i][:, 512 * w8p:512 * w8p + 1024], kp[:, 0:1024],
                qb[4 + i][:, 0:1], None, Alu.add)

        def q_chain2(i, par):
            # both query windows in one psum alloc + one bias-cast
            qp = pool_ab[par].tile([128, 1024], f32, name="qp2",
                                   tag="sA" if par == 0 else "sB")
            for w in range(2):
                for kk in range(2):
                    nc.tensor.matmul(
                        qp[:, 512 * w:512 * w + 512],
                        wq_pair(kk, 128 * i, 128), xn_pair(kk, 512 * w, 512),
                        start=(kk == 0), stop=(kk == 1), perf_mode=DR)
            nc.vector.tensor_scalar(
                qsb[i][:, 0:1024], qp[:, 0:1024], qb[i][:, 0:1], None, Alu.add)

        def k_chain(i, w8, par, sc=False):
            kp = pool_ab[par].tile([128, 512], f32, name="kp",
                                   tag="sA" if par == 0 else "sB")
            for kk in range(2):
                nc.tensor.matmul(
                    kp[:], wq_pair(kk, 512 + 128 * i, 128),
                    xn_pair(kk, 512 * w8, 512),
                    start=(kk == 0), stop=(kk == 1), perf_mode=DR)
            if sc:
                nc.scalar.activation(ksb[i][:, 512 * w8:512 * w8 + 512], kp[:],
                                     Act.Identity, bias=qb[4 + i][:, 0:1])
            else:
                nc.vector.tensor_scalar(
                    ksb[i][:, 512 * w8:512 * w8 + 512], kp[:], qb[4 + i][:, 0:1],
                    None, Alu.add)

        def v_chain2(mt, par):
            # two m-tiles per psum alloc: halves the pool round-trips that
            # serialize the early-period v drain against the S stream
            vp = pool_ab[par].tile([128, 1024], f32, name="vp2",
                                   tag="sA" if par == 0 else "sB")
            for sub in range(2):
                for kk in range(2):
                    nc.tensor.matmul(
                        vp[:, 512 * sub:512 * sub + 512],
                        xn_pair(kk, 128 * (mt + sub), 128),
                        wq_pair(kk, 1024, 512),
                        start=(kk == 0), stop=(kk == 1), perf_mode=DR)
            dst = vT[:, H * VX * mt:H * VX * (mt + 2)].rearrange(
                "p (m h x) -> p m h x", m=2, x=VX)[:, :, :, 0:64]
            srcv = vp[:].rearrange("p (m h x) -> p m h x", m=2, x=64)
            nc.vector.tensor_copy(dst, srcv)

        def v_chain(mt, par, sc=False):
            # out vp[128 keys, 512 vchan]; bias deferred to emit_normalize
            vp = pool_ab[par].tile([128, 512], f32, name="vp",
                                   tag="sA" if par == 0 else "sB")
            for kk in range(2):
                nc.tensor.matmul(
                    vp[:], xn_pair(kk, 128 * mt, 128), wq_pair(kk, 1024, 512),
                    start=(kk == 0), stop=(kk == 1), perf_mode=DR)
            dst = vT[:, H * VX * mt:H * VX * (mt + 1)].rearrange(
                "p (h x) -> p h x", x=VX)[:, :, 0:64]
            srcv = vp[:].rearrange("p (h x) -> p h x", x=64)
            # pure cast; prefix v casts may use ScalarE (idle pre-stream)
            if sc:
                nc.scalar.activation(dst, srcv, Act.Copy)
            else:
                nc.vector.tensor_copy(dst, srcv)

        def proj_chain(i, w, par, kks=(0, 1), chunks=None, partial=None,
                       combine=None, combine2=None):
            py = pool_ab[par].tile([128, 512], f32, name="py",
                                   tag="sA" if par == 0 else "sB")
            if chunks is None:
                for n_, kk in enumerate(kks):
                    nc.tensor.matmul(
                        py[:], wp_pair(kk, 128 * i, 128),
                        attn_pair(kk, 512 * w, 512),
                        start=(n_ == 0), stop=(n_ == len(kks) - 1), perf_mode=DR)
            else:
                # single-chunk fp8 matmuls (contraction 128 each, no DR)
                for n_, ck in enumerate(chunks):
                    nc.tensor.matmul(
                        py[:], wT8p[:, C * ck + 128 * i:C * ck + 128 * i + 128],
                        attn8[:, NQ * ck + 512 * w:NQ * ck + 512 * w + 512],
                        start=(n_ == 0), stop=(n_ == len(chunks) - 1))
            if partial is not None:
                nc.vector.tensor_copy(partial[:], py[:])
                return
            yo = yo_pool.tile([128, 512], f32, name="yo", tag="yo")
            nc.vector.scalar_tensor_tensor(
                yo[:], py[:], pb[i][:, 0:1], xres[i][:, 512 * w:512 * w + 512],
                Alu.add, Alu.add)
            if combine is not None:
                nc.vector.tensor_tensor(yo[:], yo[:], combine[:], Alu.add)
            if combine2 is not None:
                nc.vector.tensor_tensor(yo[:], yo[:], combine2[:], Alu.add)
            nc.sync.dma_start(y[128 * i:128 * i + 128, 512 * w:512 * w + 512], yo[:])

        # prefix: k tile 0 (all cols; dup half1 needs it by period 5),
        # k tile 1 cols 0-2047, both q windows; the rest stream as fillers
        k_chain2(0, 0, 0)
        k_chain2(1, 0, 1)
        k_chain2(0, 2, 0)
        k_chain2(1, 2, 1)
        k_chain2(0, 4, 0)
        k_chain2(0, 6, 1)
        for i in range(T):
            q_chain2(i, i % 2)

        # ---------------- phase 4: attention (flat pipelined stream) ------
        # Global stream of periods over (window, pair, group). PV runs one
        # period behind S/exp; pair normalize is deferred into the next
        # pair's first period; filler chains (k tiles 1-3, window-1 q,
        # window-0 proj) are emitted on alternate periods.
        def gsize(r):
            return 3 if r < NGRP - 1 else MT - 3 * (NGRP - 1)

        periods = [(w, p, r) for w in range(W) for p in range(4)
                   for r in range(NGRP)]
        pair_state = {}
        dup_state = {}

        def prep_dup(w, p, half):
            # swapped-row copies: kdup/qdup rows 64-127 hold head h0's data,
            # rows 0-63 hold h1's, so alternate S matmuls can run on disjoint
            # PE row halves (row-tiling concurrency). Pure relocation - no
            # numerical change. SBUF->SBUF DMA on otherwise idle engines.
            # Split in column halves so each DMA is emitted strictly after
            # the filler chains producing its source columns.
            if half == 0:
                kd = dup_pool.tile([128, N], bf16, name="kdup", tag="kdup")
                qd = dup_pool.tile([128, NQ], bf16, name="qdup", tag="qdup")
                dup_state[(w, p)] = (kd, qd)
                # q: only this pair's window columns are ever read
                nc.sync.dma_start(qd[64:128, 512 * w:512 * w + 512],
                                  qsb[p][0:64, 512 * w:512 * w + 512])
                nc.sync.dma_start(qd[0:64, 512 * w:512 * w + 512],
                                  qsb[p][64:128, 512 * w:512 * w + 512])
                lo, hi = 0, 2048
            else:
                kd, qd = dup_state[(w, p)]
                lo, hi = 2048, N
            nc.sync.dma_start(kd[64:128, lo:hi], ksb[p][0:64, lo:hi])
            nc.sync.dma_start(kd[0:64, lo:hi], ksb[p][64:128, lo:hi])

        def jmax(r):
            # last complete m-PAIR after exp group r: pairs j with 2j+1 <= 3r+2
            return (3 * r + 1) // 2

        def emit_pv(w, p, r):
            # fp8 DoubleRow: one matmul per m-PAIR (2 key-tiles, 2x rate)
            pvs, ps_t = pair_state[(w, p)]
            if pvs[0] is None:
                for hh in range(2):
                    pvs[hh] = psum_pv.tile([128, 512], f32, name=f"pv{hh}", tag="pv")
            j0 = jmax(r - 1) + 1 if r > 0 else 0
            for hh in range(2):
                h = 2 * p + hh
                pst = ps_t[hh]
                for j in range(j0, jmax(r) + 1):
                    s = (2 * j) % PRING
                    vpair = vT[:, 2 * H * VX * j:2 * H * VX * (j + 1)].rearrange(
                        "p (two h x) -> p two h x", two=2, x=VX)[:, :, h, :]
                    ppair = pst[:, 512 * s:512 * s + 1024].rearrange(
                        "p (two n) -> p two n", two=2)
                    nc.tensor.matmul(
                        pvs[hh][0:VX, :], vpair, ppair,
                        start=(j == 0), stop=(j == MT // 2 - 1),
                        perf_mode=DR)

        def emit_normalize(w, p, last=False):
            pvs, _ = pair_state[(w, p)]
            for hh in range(2):
                h = 2 * p + hh
                kt, prr = h // 2, 64 * (h % 2)
                pvc = rec_pool.tile([65, 512], f32, name="pvc", tag="pvc")
                nc.vector.tensor_copy(pvc[0:65, :], pvs[hh][0:65, :])
                dnm = rec_pool.tile([1, 512], f32, name="dnm", tag="dnm")
                nc.vector.tensor_copy(dnm[0:1, :], pvc[64:65, :])
                rec = rec_pool.tile([1, 512], f32, name="rec", tag="rec")
                rscr = rec_pool.tile([1, 512], f32, name="rscr", tag="rscr")
                nc.vector.reciprocal_approx_accurate(
                    rec[0:1, :], dnm[0:1, :], rscr[0:1, :])
                if last:
                    # tail: PE one-row broadcast (gpsimd drain costs ~4us)
                    bps = psum_pv.tile([64, 512], f32, name="bps", tag="pv")
                    nc.tensor.matmul(bps[0:64, :], ones_row[0:1, 0:64],
                                     rec[0:1, :], start=True, stop=True)
                    bcs = bps
                else:
                    # gpsimd partition-broadcast: idle engine, pure SBUF
                    bcs = rec_pool.tile([64, 512], f32, name="bcs", tag="bcs")
                    nc.gpsimd.partition_broadcast(bcs[0:64, :], rec[0:1, :])
                tmp = rec_pool.tile([64, 512], f32, name="tmp", tag="tmp")
                nc.vector.tensor_tensor(
                    tmp[0:64, :], pvc[0:64, :], bcs[0:64, :], Alu.mult)
                # + v-bias (per v-channel == partition here), cast fp8
                nc.vector.tensor_scalar(
                    attn8[prr:prr + 64, NQ * kt + 512 * w:NQ * kt + 512 * w + 512],
                    tmp[0:64, :], vbp[kt][prr:prr + 64, 0:1], None, Alu.add)

        # filler schedule: (earliest_period, closure); one pop on EVEN
        # periods, always from pool B (its next S alloc has ~2x more slack
        # than pool A's, so the filler's drain never delays the exp stream).
        # Emission deadlines: ksb[2] before period 22, ksb[3] before 33,
        # window-1 q before 44, window-0 proj after normalize(w0,p3) at 44.
        fillers = []
        for n_, w8p in enumerate((4, 6)):           # ksb[1] cols 2048+: by the
            fillers.append((5 + 2 * n_,             # pair-1 dup at g=10
                            lambda w8p=w8p: k_chain2(1, w8p, 1)))
        for i in range(2, T):                       # ksb[2..3]
            for n_, w8p in enumerate((0, 2, 4, 6)):
                fillers.append((16 * (i - 2) + 2 + 4 * n_,
                                lambda i=i, w8p=w8p: k_chain2(i, w8p, 1)))
        fillers.sort(key=lambda f: f[0])
        for i in range(T):                          # window-0 proj: pops 46..52
            fillers.append((46 + 2 * i, lambda i=i: proj_chain(i, 0, 1)))
        for i in range(T):                          # w1 proj half (pairs 0-1)
            fillers.append((70 + 2 * i, lambda i=i:
                            proj_chain(i, 1, 1, kks=(0,), partial=yh[i])))

        def pre_res(i):
            # yh[i] + proj bias + residual, pre-fused so the tail is one add
            nc.vector.scalar_tensor_tensor(
                yh[i][:], yh[i][:], pb[i][:, 0:1], xres[i][:, 512:1024],
                Alu.add, Alu.add)
        for i in range(T):
            fillers.append((80 + i, lambda i=i: pre_res(i)))
        fillers.reverse()   # pop from the end

        prep_dup(0, 0, 0)
        prep_dup(0, 0, 1)
        for mt in range(8):
            v_chain(mt, mt % 2)
        vq = list(range(8, MT, 2))  # v m-pairs: drained 2-per-period below
        for g, (w, p, r) in enumerate(periods):
            if g == 1:
                # proj weight prep: PE transposes must precede the first PV
                # matmul in the queue (transpose inside an open PV
                # accumulation group corrupts PSUM)
                for i_ in range(4):
                    prep_w2(i_)
            gs = gsize(r)
            if r == 0:
                pair_state[(w, p)] = (
                    [None, None],
                    [psb_pool.tile([128, 512 * PRING], fp8, name=f"ps{hh}",
                                   tag="ps") for hh in range(2)])
            pvs, ps_t = pair_state[(w, p)]
            kd, qd = dup_state[(w, p)]
            for hh in range(2):
                sp = pool_ab[hh].tile([128, 512 * gs], f32, name=f"sp{hh}",
                                      tag="sA" if hh == 0 else "sB")
                for j in range(gs):
                    m = 3 * r + j
                    # even j: head's native row half; odd j: the swapped copy
                    # on the opposite half -> adjacent matmuls use disjoint
                    # PE row groups and run concurrently
                    if j % 2 == 0:
                        pr, kt_, qt_ = 64 * hh, ksb[p], qsb[p]
                    else:
                        pr, kt_, qt_ = 64 * (1 - hh), kd, qd
                    nc.tensor.matmul(
                        sp[:, 512 * j:512 * j + 512],
                        kt_[pr:pr + 64, 128 * m:128 * m + 128],
                        qt_[pr:pr + 64, 512 * w:512 * w + 512],
                        start=True, stop=True)
                s0 = (3 * r) % PRING
                nc.scalar.activation(
                    ps_t[hh][:, 512 * s0:512 * s0 + 512 * gs],
                    sp[:, 0:512 * gs], Act.Exp, scale=0.125,
                    bias=ebias[:, 0:1])
                # PV of the previous period goes between the two S blocks
                # (h0 part) and after them (h1 part)
                if hh == 0:
                    if g > 0:
                        pw, pp, prr_ = periods[g - 1]
                        emit_pv(pw, pp, prr_)
            if g > 0 and periods[g - 1][2] == NGRP - 1:
                emit_normalize(*periods[g - 1][:2])
            for _ in range(2):
                # v(m,m+1) must land before PV pair m/2 fires at period
                # ceil((m-1)/3)+1
                if vq and g >= 1:
                    mt = vq.pop(0)
                    v_chain2(mt, (mt // 2) % 2)
                else:
                    break
            if fillers and fillers[-1][0] <= g:
                fillers.pop()[1]()
            if r == 5 and g + 6 < len(periods):
                prep_dup(*periods[g + 6][:2], 0)
            if r == NGRP - 1 and g + 1 < len(periods):
                prep_dup(*periods[g + 1][:2], 1)

        # tail: last period's PV, last normalize, window-1 proj second half
        emit_pv(*periods[-1])
        emit_normalize(W - 1, 3, last=True)
        while fillers:
            fillers.pop()[1]()
        for i in range(T):
            py = pool_ab[i % 2].tile([128, 512], f32, name="py",
                                     tag="sA" if i % 2 == 0 else "sB")
            nc.tensor.matmul(py[:], wp_pair(1, 128 * i, 128),
                             attn_pair(1, 512, 512),
                             start=True, stop=True, perf_mode=DR)
            yo = yo_pool.tile([128, 512], f32, name="yo", tag="yo")
            nc.vector.tensor_tensor(yo[:], py[:], yh[i][:], Alu.add)
            nc.sync.dma_start(y[128 * i:128 * i + 128, 512:1024], yo[:])


def _build():
    import concourse.tile as tile
    from concourse import bacc, mybir

    nc = bacc.Bacc("TRN2", target_bir_lowering=False, debug=False)
    f32 = mybir.dt.float32
    io = {
        "xb": nc.dram_tensor("xb", [C, N], f32, kind="ExternalInput").ap(),
        "qkvw": nc.dram_tensor("qkvw", [3 * C, C], f32, kind="ExternalInput").ap(),
        "qkvb": nc.dram_tensor("qkvb", [3 * C], f32, kind="ExternalInput").ap(),
        "projw": nc.dram_tensor("projw", [C, C], f32, kind="ExternalInput").ap(),
        "projb": nc.dram_tensor("projb", [C], f32, kind="ExternalInput").ap(),
        "nw": nc.dram_tensor("nw", [C], f32, kind="ExternalInput").ap(),
        "nb": nc.dram_tensor("nb", [C], f32, kind="ExternalInput").ap(),
        "cid": nc.dram_tensor("cid", [128, 128], mybir.dt.float32,
                              kind="ExternalInput").ap(),
        "cind": nc.dram_tensor("cind", [128, 2], f32, kind="ExternalInput").ap(),
        "cindT": nc.dram_tensor("cindT", [2, 128], f32, kind="ExternalInput").ap(),
        "y": nc.dram_tensor("y", [C, NQ], f32, kind="ExternalOutput").ap(),
    }
    with tile.TileContext(nc) as tc:
        _emit(tc, io)
    nc.compile()
    return nc


def get_compiled():
    global _COMPILED
    if _COMPILED is None:
        _COMPILED = _build()
    return _COMPILED


def make_in_maps(x, norm_w, norm_b, qkv_w, qkv_b, proj_w, proj_b):
    import ml_dtypes

    xf = np.ascontiguousarray(np.asarray(x, np.float32)).reshape(2, C, N)
    ind = np.zeros((128, 2), np.float32)
    ind[0:64, 0] = 1.0
    ind[64:128, 1] = 1.0
    shared = {
        "cid": np.eye(128, dtype=np.float32),
        "cind": ind,
        "cindT": np.ascontiguousarray(ind.T),
        "qkvw": np.ascontiguousarray(np.asarray(qkv_w, np.float32)),
        "qkvb": np.ascontiguousarray(np.asarray(qkv_b, np.float32)),
        "projw": np.ascontiguousarray(np.asarray(proj_w, np.float32)),
        "projb": np.ascontiguousarray(np.asarray(proj_b, np.float32)),
        "nw": np.ascontiguousarray(np.asarray(norm_w, np.float32)),
        "nb": np.ascontiguousarray(np.asarray(norm_b, np.float32)),
    }
    in_maps = []
    for core in range(8):
        bi, qs = core // 4, core % 4
        # rotate so this core's queries are always columns [0:NQ)
        xroll = np.concatenate(
            [xf[bi][:, qs * NQ:], xf[bi][:, :qs * NQ]], axis=1)
        m = dict(shared)
        m["xb"] = np.ascontiguousarray(xroll)
        in_maps.append(m)
    return in_maps


def assemble(results, x):
    y = np.zeros((2, C, N), np.float32)
    for core in range(8):
        bi, qs = core // 4, core % 4
        y[bi][:, qs * NQ:(qs + 1) * NQ] = results[core]["y"]
    return y.reshape(x.shape)


def kernel(x, norm_w, norm_b, qkv_w, qkv_b, proj_w, proj_b, **_ignored):
    from concourse import bass_utils

    nc = get_compiled()
    in_maps = make_in_maps(x, norm_w, norm_b, qkv_w, qkv_b, proj_w, proj_b)
    res = bass_utils.run_bass_kernel_spmd(nc, in_maps, core_ids=list(range(8)))
    return assemble(res.results, np.asarray(x))
